# revision 34
# baseline (speedup 1.0000x reference)
"""AngleProto loss kernel for 8 TRN2 NeuronCores.

x: [16384, 512] f32, interleaved (anchor, positive) rows.
loss = mean_i( logsumexp_j(w*cos(pos_i, anc_j) + b) - (w*cos(pos_i, anc_i) + b) )

Sharding: core c owns pairs [c*1024, (c+1)*1024) == x rows [c*2048, (c+1)*2048)
(a contiguous block).  Per core: normalize own rows (sumsq fused on DVE via
tensor_tensor_reduce, 1/sqrt via exp(-0.5*ln) on ACT -- one table set), cast to
fp8 (x16 scale), transpose via TensorE (PE is idle pre-GEMM), all-gather the
transposed anchors (0.5 MB/rank), then a DoubleRow fp8 GEMM over the
[1024, 8192] row-block with a fused exp/row-sum epilogue on ACT (accum_out) --
the logits never touch HBM.  Softmax max-subtraction is skipped: logits are
bounded (|cos|<=1), so exp() cannot overflow in f32.  The diagonal logit comes
from fused pair-dots of the core's own rows.  Per-row losses [128, 8] are
DMA'd out; the host sums across cores / 8192.
"""

import sys

import numpy as np

for _p in ("/opt/trn_rl_repo",):
    if _p not in sys.path:
        sys.path.append(_p)

import concourse.bass as bass  # noqa: E402
import concourse.mybir as mybir  # noqa: E402
import concourse.tile as tile  # noqa: E402
from concourse import bacc  # noqa: E402
from concourse.bass_utils import run_bass_kernel_spmd  # noqa: E402

N_CORES = 8
D = 512
N_PAIRS = 8192
ROWS = N_PAIRS // N_CORES  # 1024 pairs per core
RB = ROWS // 128  # 8 row-blocks of 128
KT = D // 128  # 4 contraction tiles of 128
F32 = mybir.dt.float32
BF16 = mybir.dt.bfloat16
FP8 = mybir.dt.float8e4
AF = mybir.ActivationFunctionType
ALU = mybir.AluOpType
AX = mybir.AxisListType

# fp8 operand pre-scale: normalized elements are ~N(0, 1/512); x16 moves them
# out of the e4m3 subnormal range. dots scale by 16^2; folded into exp scale.
FP8_SCALE = 16.0


def _gemm_tail(nc, R, fp8, variant, gemm_scale, bias_t, ls_all,
               ptt, atr, psum_pool, esc_pool, row_sum=None):
    # psum tile [128, 2048] spans 4 banks: ranks 4g..4g+3, column-half h.
    # All h=0 groups run first -- they only need the first all-gather half.
    for h in range(2):
        for m in range(RB):
            for g in range(2):
                gi = 2 * h + g
                ps = psum_pool.tile([128, 2048], F32, name=f"ps{m}_{gi}{R}",
                                    tag="ps")
                if fp8:
                    # DoubleRow: contract a pair of k-tiles per matmul.
                    # lhsT [128, 2, 128] (free 256 -> out part 128),
                    # rhs [128, 2, 512] (free 1024 -> out free 512).
                    for ktp in range(KT // 2):
                        lhsT = ptt[m][:, 2 * ktp : 2 * ktp + 2, :]
                        for j in range(4):
                            r = 4 * g + j
                            nc.tensor.matmul(
                                ps[:, j * 512 : (j + 1) * 512],
                                lhsT=lhsT,
                                rhs=atr[r][:, 2 * ktp : 2 * ktp + 2,
                                           h * 512 : (h + 1) * 512],
                                start=(ktp == 0),
                                stop=(ktp == KT // 2 - 1),
                                perf_mode=mybir.MatmulPerfMode.DoubleRow,
                            )
                else:
                    for kt in range(KT):
                        lhsT = ptt[m][:, kt, :]
                        for j in range(4):
                            r = 4 * g + j
                            nc.tensor.matmul(
                                ps[:, j * 512 : (j + 1) * 512],
                                lhsT=lhsT,
                                rhs=atr[r][:, kt, h * 512 : (h + 1) * 512],
                                start=(kt == 0),
                                stop=(kt == KT - 1),
                            )
                if variant in ("noexp", "mmpure"):
                    # Ablation: tiny DVE read instead of the ACT exp drain.
                    nc.vector.tensor_reduce(
                        ls_all[:, m, gi : gi + 1], ps[:, 0:16], axis=AX.X,
                        op=ALU.add,
                    )
                else:
                    esc = esc_pool.tile([128, 2048], BF16, name=f"e{m}_{gi}{R}",
                                        tag="esc")
                    nc.scalar.activation(
                        esc,
                        ps,
                        AF.Exp,
                        bias=bias_t,
                        scale=gemm_scale,
                        accum_out=ls_all[:, m, gi : gi + 1],
                    )
                if h == 1 and g == 1 and row_sum is not None:
                    # drain this row-block's sum as soon as it completes
                    nc.vector.tensor_reduce(
                        row_sum[:, m : m + 1], ls_all[:, m, :], axis=AX.X,
                        op=ALU.add,
                    )


def _emit(nc, pools, w: float, b: float, rep: int, fp8: bool,
          variant: str = "full"):
    pin, scratch, tp, esc_pool, psum_pool, dram = pools
    R = f"_r{rep}"
    P2 = rep % 2
    gdt = FP8 if fp8 else BF16
    cast_scale = FP8_SCALE if fp8 else 1.0
    gemm_scale = float(w) / (FP8_SCALE * FP8_SCALE) if fp8 else float(w)
    x_in = nc._x_in
    out = nc._out
    xp = x_in.ap().rearrange("(n two) d -> two n d", two=2)
    gemm_only = variant in ("mmonly", "mmpure")

    # DRAM bounce buffers for the collective, split in two halves of the
    # core's anchor rows so the gather pipelines: the GEMM's first column-half
    # only needs AG half 0. Layout [kt, c, row] so fills read contiguous runs.
    at_own = [
        dram.tile([KT, 128, ROWS // 2], gdt, name=f"at_own{hh}{R}",
                  tag=f"at_own{hh}_{P2}")
        for hh in range(2)
    ]
    ag = [
        dram.tile([N_CORES, KT, 128, ROWS // 2], gdt, name=f"ag{hh}{R}",
                  tag=f"ag{hh}_{P2}", addr_space="Shared")
        for hh in range(2)
    ]

    def pint(shape, dtype, nm):
        return pin.tile(shape, dtype, name=f"{nm}{R}", tag=nm)

    an_ss = pint([128, RB], F32, "an_ss")
    pn_ss = pint([128, RB], F32, "pn_ss")
    an_ln = pint([128, RB], F32, "an_ln")
    pn_ln = pint([128, RB], F32, "pn_ln")
    an_inv = pint([128, RB], F32, "an_inv")
    pn_inv = pint([128, RB], F32, "pn_inv")
    diag = pint([128, RB], F32, "diag")
    ls_all = pint([128, RB, 4], F32, "ls_all")
    row_sum = pint([128, RB], F32, "row_sum")
    lse = pint([128, RB], F32, "lse")
    dlog = pint([128, RB], F32, "dlog")
    loss_t = pint([128, RB], F32, "loss_t")

    # Bias constant for the fused exp(w*x + b) activation
    bias_t = pint([128, 1], F32, "bias_t")
    nc.vector.memset(bias_t, float(b))

    def tail():
        nc.scalar.activation(lse, row_sum, AF.Ln)
        nc.vector.tensor_scalar(
            dlog, diag, float(w), float(b), op0=ALU.mult, op1=ALU.add
        )
        nc.vector.tensor_tensor(loss_t, lse, dlog, ALU.subtract)
        nc.scalar.dma_start(out.ap(), loss_t)

    if variant == "empty":
        nc.vector.memset(loss_t, 0.0)
        nc.scalar.dma_start(out.ap(), loss_t)
        return

    if gemm_only:
        # Ablation: GEMM/exp pipeline only, on zeroed operand tiles.
        ptt = [
            pin.tile([128, KT, 128], gdt, name=f"ptt{rb}{R}",
                     tag=f"ptt{rb}_{P2}")
            for rb in range(RB)
        ]
        atr = [
            pin.tile([128, KT, ROWS], gdt, name=f"atr{r}{R}",
                     tag=f"atr{r}_{P2}")
            for r in range(N_CORES)
        ]
        nc.vector.memset(diag, 0.0)
        for t in ptt:
            nc.vector.memset(t, 0.0)
        for t in atr:
            nc.vector.memset(t, 0.0)
        _gemm_tail(nc, R, fp8, variant, gemm_scale, bias_t, ls_all,
                   ptt, atr, psum_pool, esc_pool, row_sum=row_sum)
        tail()
        return

    # Raw rows staged on-chip: [128, rb, d]
    xa_all = pint([128, RB, D], F32, "xa_all")
    xp_all = pint([128, RB, D], F32, "xp_all")
    xa_src = xp[0].rearrange("(rb p) d -> p rb d", p=128)
    xp_src = xp[1].rearrange("(rb p) d -> p rb d", p=128)

    # Normalized bf16 rows
    xna_all = pint([128, RB, D], BF16, "xna_all")
    xnp_all = pint([128, RB, D], BF16, "xnp_all")

    # ---- anchors first (they gate the all-gather): load (SP ring) ->
    # fused sumsq (DVE) -> 1/norm = exp(-0.5*ln(ss)) (ACT) -> normalize ->
    # TensorE transpose -> fp8 cast -> bounce to DRAM.
    load_eng = nc.gpsimd if variant == "ringiso" else nc.sync
    bounce_eng = nc.scalar if variant == "ringiso" else nc.sync
    for hh in range(2):
        load_eng.dma_start(
            xa_all[:, hh * 4 : (hh + 1) * 4, :],
            xa_src[:, hh * 4 : (hh + 1) * 4, :],
        )
        for rb in range(hh * 4, (hh + 1) * 4):
            sqa = scratch.tile([128, D], F32, name=f"sqa{rb}{R}", tag="sq")
            nc.scalar.activation(
                sqa, xa_all[:, rb, :], AF.Square,
                accum_out=an_ss[:, rb : rb + 1],
            )
    nc.scalar.activation(an_ln, an_ss, AF.Ln)
    nc.scalar.activation(an_inv, an_ln, AF.Exp, scale=-0.5)
    for hh in range(2):
        for rb in range(hh * 4, (hh + 1) * 4):
            nc.vector.tensor_scalar_mul(
                xna_all[:, rb, :], xa_all[:, rb, :], an_inv[:, rb : rb + 1]
            )
            tpa = tp.tile([128, KT, 128], BF16, name=f"tpa{rb}{R}",
                          tag=f"tpa{rb % 4}")
            nc.sync.dma_start_transpose(tpa, xna_all[:, rb, :])
            ab = scratch.tile([128, KT, 128], gdt, name=f"ab{rb}{R}", tag="ab")
            nc.vector.tensor_scalar_mul(ab, tpa, cast_scale)
            rbl = rb - hh * 4
            bounce_eng.dma_start(
                at_own[hh][:, :, rbl * 128 : (rbl + 1) * 128].rearrange(
                    "kt c p -> c kt p"),
                ab,
            )
        # all-gather this half of the transposed anchors (0.25 MB/rank fp8)
        if variant != "noag":
            nc.gpsimd.collective_compute(
                "AllGather",
                ALU.bypass,
                replica_groups=[list(range(N_CORES))],
                ins=[at_own[hh].opt()],
                outs=[ag[hh].opt()],
            )

    # ---- positives (overlap the AG): loads on the ACT ring.
    ptt = []
    for hh in range(2):
        (nc.gpsimd if variant == "ringiso" else nc.scalar).dma_start(
            xp_all[:, hh * 4 : (hh + 1) * 4, :],
            xp_src[:, hh * 4 : (hh + 1) * 4, :],
        )
        for rb in range(hh * 4, (hh + 1) * 4):
            sqp = scratch.tile([128, D], F32, name=f"sqp{rb}{R}", tag="sq")
            nc.scalar.activation(
                sqp, xp_all[:, rb, :], AF.Square,
                accum_out=pn_ss[:, rb : rb + 1],
            )
    nc.scalar.activation(pn_ln, pn_ss, AF.Ln)
    nc.scalar.activation(pn_inv, pn_ln, AF.Exp, scale=-0.5)
    if True:
        for rb in range(RB):
            nc.vector.tensor_scalar_mul(
                xnp_all[:, rb, :], xp_all[:, rb, :], pn_inv[:, rb : rb + 1]
            )
            tpp = tp.tile([128, KT, 128], BF16, name=f"tpp{rb}{R}",
                          tag=f"tpp{rb % 4}")
            nc.sync.dma_start_transpose(tpp, xnp_all[:, rb, :])
            t = pin.tile([128, KT, 128], gdt, name=f"ptt{rb}{R}",
                         tag=f"ptt{rb}_{P2}")
            nc.vector.tensor_scalar_mul(t, tpp, cast_scale)
            ptt.append(t)

    # ---- diagonal: cos(pos_i, anc_i) from own normalized rows
    dprod = scratch.tile([128, RB, D], BF16, name=f"dprod{R}", tag="dprod")
    nc.vector.tensor_tensor(dprod, xna_all, xnp_all, ALU.mult)
    nc.vector.tensor_reduce(diag, dprod, axis=AX.X, op=ALU.add)

    # ---- fill gathered anchor-T tiles: atr[r] = [c, kt, 1024] (ACT ring)
    atr = []
    for r in range(N_CORES):
        t = pin.tile([128, KT, ROWS], gdt, name=f"atr{r}{R}", tag=f"atr{r}_{P2}")
        for hh in range(2):
            src = at_own[hh] if variant == "noag" else ag[hh][r]
            nc.scalar.dma_start(
                t[:, :, hh * 512 : (hh + 1) * 512],
                src.rearrange("kt c p -> c kt p"),
            )
        atr.append(t)

    if variant == "nomm":
        for r in range(N_CORES):
            nc.vector.tensor_reduce(
                ls_all[:, r, 0:1], atr[r][:, 0, 0:16], axis=AX.X, op=ALU.add,
            )
        for m in range(RB):
            nc.vector.tensor_reduce(
                ls_all[:, m, 1:2], ptt[m][:, 0, 0:16], axis=AX.X, op=ALU.add,
            )
        nc.vector.tensor_reduce(row_sum, ls_all, axis=AX.X, op=ALU.add)
    else:
        _gemm_tail(nc, R, fp8, variant, gemm_scale, bias_t, ls_all,
                   ptt, atr, psum_pool, esc_pool, row_sum=row_sum)

    # ---- per-row loss = ln(sum_j exp(logit)) - (w*diag + b)
    tail()


def build(w: float, b: float, repeat: int = 1, fp8: bool = True,
          variant: str = "full"):
    nc = bacc.Bacc(
        "TRN2",
        target_bir_lowering=False,
        debug=False,
        enable_asserts=False,
        num_devices=N_CORES,
    )
    nc._x_in = nc.dram_tensor("x_own", [2 * ROWS, D], F32, kind="ExternalInput")
    nc._out = nc.dram_tensor("out", [128, RB], F32, kind="ExternalOutput")

    with tile.TileContext(nc) as tc:
        with (
            tc.tile_pool(name="pin", bufs=1) as pin,
            tc.tile_pool(name="scratch", bufs=3) as scratch,
            tc.tile_pool(name="tp", bufs=4) as tp,
            tc.tile_pool(name="esc", bufs=3) as esc_pool,
            tc.tile_pool(name="psum", bufs=2, space="PSUM") as psum_pool,
            tc.tile_pool(name="dram", bufs=1, space="DRAM") as dram,
        ):
            pools = (pin, scratch, tp, esc_pool, psum_pool, dram)
            for rep in range(repeat):
                _emit(nc, pools, w, b, rep, fp8, variant)

    nc.compile()
    return nc


_CACHE: dict = {}


def _get_nc(w: float, b: float, repeat: int = 1, fp8: bool = True,
            variant: str = "full"):
    key = (w, b, repeat, fp8, variant)
    if key not in _CACHE:
        _CACHE[key] = build(w, b, repeat, fp8, variant)
    return _CACHE[key]


def run(x: np.ndarray, w: float, b: float, repeat: int = 1, fp8: bool = True,
        variant: str = "full", **spmd_kwargs):
    nc = _get_nc(w, b, repeat, fp8, variant)
    x = np.ascontiguousarray(np.asarray(x, dtype=np.float32))
    in_maps = [
        {"x_own": x[c * 2 * ROWS : (c + 1) * 2 * ROWS]} for c in range(N_CORES)
    ]
    res = run_bass_kernel_spmd(nc, in_maps, core_ids=list(range(N_CORES)),
                               **spmd_kwargs)
    total = sum(float(r["out"].astype(np.float64).sum()) for r in res.results)
    return np.float32(total / N_PAIRS), res


def kernel(x, w, b):
    wf = float(np.asarray(w))
    bf = float(np.asarray(b))
    loss, _ = run(np.asarray(x), wf, bf)
    return loss


# revision 38
# speedup vs baseline: 1.8394x; 1.8394x over previous
"""AngleProto loss kernel for 8 TRN2 NeuronCores.

x: [16384, 512] f32, interleaved (anchor, positive) rows.
loss = mean_i( logsumexp_j(w*cos(pos_i, anc_j) + b) - (w*cos(pos_i, anc_i) + b) )

Sharding: core c owns pairs [c*1024, (c+1)*1024) == x rows [c*2048, (c+1)*2048)
(a contiguous block).  Per core: normalize own rows (sumsq fused on DVE via
tensor_tensor_reduce, 1/sqrt via exp(-0.5*ln) on ACT -- one table set), cast to
fp8 (x16 scale), transpose via TensorE (PE is idle pre-GEMM), all-gather the
transposed anchors (0.5 MB/rank), then a DoubleRow fp8 GEMM over the
[1024, 8192] row-block with a fused exp/row-sum epilogue on ACT (accum_out) --
the logits never touch HBM.  Softmax max-subtraction is skipped: logits are
bounded (|cos|<=1), so exp() cannot overflow in f32.  The diagonal logit comes
from fused pair-dots of the core's own rows.  Per-row losses [128, 8] are
DMA'd out; the host sums across cores / 8192.
"""

import sys

import numpy as np

for _p in ("/opt/trn_rl_repo",):
    if _p not in sys.path:
        sys.path.append(_p)

import concourse.bass as bass  # noqa: E402
import concourse.mybir as mybir  # noqa: E402
import concourse.tile as tile  # noqa: E402
from concourse import bacc  # noqa: E402
from concourse.bass_utils import run_bass_kernel_spmd  # noqa: E402

N_CORES = 8
D = 512
N_PAIRS = 8192
ROWS = N_PAIRS // N_CORES  # 1024 pairs per core
RB = ROWS // 128  # 8 row-blocks of 128
KT = D // 128  # 4 contraction tiles of 128
F32 = mybir.dt.float32
BF16 = mybir.dt.bfloat16
FP8 = mybir.dt.float8e4
AF = mybir.ActivationFunctionType
ALU = mybir.AluOpType
AX = mybir.AxisListType

# fp8 operand pre-scale: normalized elements are ~N(0, 1/512); x16 moves them
# out of the e4m3 subnormal range. dots scale by 16^2; folded into exp scale.
FP8_SCALE = 16.0
I32 = mybir.dt.int32


def _dve_rsqrt(nc, scratch, out, ss, nm):
    """out = 1/sqrt(ss) on DVE: 0x5f3759df bit-trick + 2 Newton iterations.
    Keeps Ln/Exp off the norm path so ACT needs only one exp-table load
    until the final LSE ln (the table selector reloads on every Ln<->Exp
    switch otherwise)."""
    shape = list(ss.shape)
    bi = scratch.tile(shape, I32, name=f"rsb{nm}", tag="rsb")
    nc.vector.tensor_scalar(bi, ss.bitcast(I32), 1, None,
                            op0=ALU.arith_shift_right)
    nc.vector.tensor_scalar(bi, bi, 0, None, op0=ALU.bitwise_not)
    nc.vector.tensor_scalar(bi, bi, 0x5F3759DF + 1, None, op0=ALU.add)
    y = bi.bitcast(F32)
    t = scratch.tile(shape, F32, name=f"rst{nm}", tag="rst")
    u = scratch.tile(shape, F32, name=f"rsu{nm}", tag="rsu")
    nc.vector.tensor_tensor(t, y, y, ALU.mult)
    nc.vector.tensor_tensor(t, t, ss, ALU.mult)
    nc.vector.tensor_scalar(u, t, -0.5, 1.5, op0=ALU.mult, op1=ALU.add)
    nc.vector.tensor_tensor(out, y, u, ALU.mult)


def _gemm_tail(nc, R, fp8, variant, gemm_scale, bias_t, ls_all,
               ptt, atr, psum_pool, esc_pool, row_sum=None):
    # psum tile [128, 2048] spans 4 banks: ranks 4g..4g+3, column-half h.
    # All h=0 groups run first -- they only need the first all-gather half.
    for h in range(2):
        for m in range(RB):
            for g in range(2):
                gi = 2 * h + g
                ps = psum_pool.tile([128, 2048], F32, name=f"ps{m}_{gi}{R}",
                                    tag="ps")
                if fp8:
                    # DoubleRow: contract a pair of k-tiles per matmul.
                    # lhsT [128, 2, 128] (free 256 -> out part 128),
                    # rhs [128, 2, 512] (free 1024 -> out free 512).
                    for ktp in range(KT // 2):
                        lhsT = ptt[m][:, 2 * ktp : 2 * ktp + 2, :]
                        for j in range(4):
                            r = 4 * g + j
                            nc.tensor.matmul(
                                ps[:, j * 512 : (j + 1) * 512],
                                lhsT=lhsT,
                                rhs=atr[r][:, 2 * ktp : 2 * ktp + 2,
                                           h * 512 : (h + 1) * 512],
                                start=(ktp == 0),
                                stop=(ktp == KT // 2 - 1),
                                perf_mode=mybir.MatmulPerfMode.DoubleRow,
                            )
                else:
                    for kt in range(KT):
                        lhsT = ptt[m][:, kt, :]
                        for j in range(4):
                            r = 4 * g + j
                            nc.tensor.matmul(
                                ps[:, j * 512 : (j + 1) * 512],
                                lhsT=lhsT,
                                rhs=atr[r][:, kt, h * 512 : (h + 1) * 512],
                                start=(kt == 0),
                                stop=(kt == KT - 1),
                            )
                if variant in ("noexp", "mmpure"):
                    # Ablation: tiny DVE read instead of the ACT exp drain.
                    nc.vector.tensor_reduce(
                        ls_all[:, m, gi : gi + 1], ps[:, 0:16], axis=AX.X,
                        op=ALU.add,
                    )
                else:
                    esc = esc_pool.tile([128, 2048], BF16, name=f"e{m}_{gi}{R}",
                                        tag="esc")
                    nc.scalar.activation(
                        esc,
                        ps,
                        AF.Exp,
                        bias=bias_t,
                        scale=gemm_scale,
                        accum_out=ls_all[:, m, gi : gi + 1],
                    )
                if h == 1 and g == 1 and row_sum is not None:
                    # drain this row-block's sum as soon as it completes
                    nc.vector.tensor_reduce(
                        row_sum[:, m : m + 1], ls_all[:, m, :], axis=AX.X,
                        op=ALU.add,
                    )


def _emit(nc, pools, w: float, b: float, rep: int, fp8: bool,
          variant: str = "full"):
    pin, scratch, tp, esc_pool, psum_pool, dram = pools
    R = f"_r{rep}"
    P2 = rep % 2
    gdt = FP8 if fp8 else BF16
    cast_scale = FP8_SCALE if fp8 else 1.0
    gemm_scale = float(w) / (FP8_SCALE * FP8_SCALE) if fp8 else float(w)
    x_in = nc._x_in
    out = nc._out
    xp = x_in.ap().rearrange("(n two) d -> two n d", two=2)
    gemm_only = variant in ("mmonly", "mmpure")

    # DRAM bounce buffers for the collective, split in two halves of the
    # core's anchor rows so the gather pipelines: the GEMM's first column-half
    # only needs AG half 0. Layout [kt, c, row] so fills read contiguous runs.
    at_own = [
        dram.tile([KT, 128, ROWS // 2], gdt, name=f"at_own{hh}{R}",
                  tag=f"at_own{hh}_{P2}")
        for hh in range(2)
    ]
    ag = [
        dram.tile([N_CORES, KT, 128, ROWS // 2], gdt, name=f"ag{hh}{R}",
                  tag=f"ag{hh}_{P2}", addr_space="Shared")
        for hh in range(2)
    ]

    def pint(shape, dtype, nm):
        return pin.tile(shape, dtype, name=f"{nm}{R}", tag=nm)

    an_ss = pint([128, RB], F32, "an_ss")
    pn_ss = pint([128, RB], F32, "pn_ss")
    an_inv = pint([128, RB], F32, "an_inv")
    pn_inv = pint([128, RB], F32, "pn_inv")
    diag = pint([128, RB], F32, "diag")
    ls_all = pint([128, RB, 4], F32, "ls_all")
    row_sum = pint([128, RB], F32, "row_sum")
    lse = pint([128, RB], F32, "lse")
    dlog = pint([128, RB], F32, "dlog")
    loss_t = pint([128, RB], F32, "loss_t")

    # Bias constant for the fused exp(w*x + b) activation
    bias_t = pint([128, 1], F32, "bias_t")
    nc.vector.memset(bias_t, float(b))

    def tail():
        nc.scalar.activation(lse, row_sum, AF.Ln)
        nc.vector.tensor_scalar(
            dlog, diag, float(w), float(b), op0=ALU.mult, op1=ALU.add
        )
        nc.vector.tensor_tensor(loss_t, lse, dlog, ALU.subtract)
        nc.scalar.dma_start(out.ap(), loss_t)

    if variant == "empty":
        nc.vector.memset(loss_t, 0.0)
        nc.scalar.dma_start(out.ap(), loss_t)
        return

    if gemm_only:
        # Ablation: GEMM/exp pipeline only, on zeroed operand tiles.
        ptt = [
            pin.tile([128, KT, 128], gdt, name=f"ptt{rb}{R}",
                     tag=f"ptt{rb}_{P2}")
            for rb in range(RB)
        ]
        atr = [
            pin.tile([128, KT, ROWS], gdt, name=f"atr{r}{R}",
                     tag=f"atr{r}_{P2}")
            for r in range(N_CORES)
        ]
        nc.vector.memset(diag, 0.0)
        for t in ptt:
            nc.vector.memset(t, 0.0)
        for t in atr:
            nc.vector.memset(t, 0.0)
        _gemm_tail(nc, R, fp8, variant, gemm_scale, bias_t, ls_all,
                   ptt, atr, psum_pool, esc_pool, row_sum=row_sum)
        tail()
        return

    # Raw rows staged on-chip: [128, rb, d]
    xa_all = pint([128, RB, D], F32, "xa_all")
    xp_all = pint([128, RB, D], F32, "xp_all")
    xa_src = xp[0].rearrange("(rb p) d -> p rb d", p=128)
    xp_src = xp[1].rearrange("(rb p) d -> p rb d", p=128)

    # Normalized bf16 rows
    xna_all = pint([128, RB, D], BF16, "xna_all")
    xnp_all = pint([128, RB, D], BF16, "xnp_all")

    # ---- anchors first (they gate the all-gather): load (SP ring) ->
    # fused sumsq (DVE) -> 1/norm = exp(-0.5*ln(ss)) (ACT) -> normalize ->
    # TensorE transpose -> fp8 cast -> bounce to DRAM.
    load_eng = nc.gpsimd if variant == "ringiso" else nc.sync
    bounce_eng = nc.scalar if variant == "ringiso" else nc.sync
    for hh in range(2):
        load_eng.dma_start(
            xa_all[:, hh * 4 : (hh + 1) * 4, :],
            xa_src[:, hh * 4 : (hh + 1) * 4, :],
        )
        for rb in range(hh * 4, (hh + 1) * 4):
            sqa = scratch.tile([128, D], F32, name=f"sqa{rb}{R}", tag="sq")
            nc.vector.tensor_tensor(sqa, xa_all[:, rb, :], xa_all[:, rb, :],
                                    ALU.mult)
            nc.vector.tensor_reduce(an_ss[:, rb : rb + 1], sqa, axis=AX.X,
                                    op=ALU.add)
    for hh in range(2):
        hs = slice(hh * 4, (hh + 1) * 4)
        _dve_rsqrt(nc, scratch, an_inv[:, hs], an_ss[:, hs], f"a{hh}{R}")
        for rb in range(hh * 4, (hh + 1) * 4):
            nc.vector.tensor_scalar_mul(
                xna_all[:, rb, :], xa_all[:, rb, :], an_inv[:, rb : rb + 1]
            )
            tpa = tp.tile([128, KT, 128], BF16, name=f"tpa{rb}{R}",
                          tag=f"tpa{rb % 4}")
            nc.sync.dma_start_transpose(tpa, xna_all[:, rb, :])
            ab = scratch.tile([128, KT, 128], gdt, name=f"ab{rb}{R}", tag="ab")
            nc.vector.tensor_scalar_mul(ab, tpa, cast_scale)
            rbl = rb - hh * 4
            bounce_eng.dma_start(
                at_own[hh][:, :, rbl * 128 : (rbl + 1) * 128].rearrange(
                    "kt c p -> c kt p"),
                ab,
            )
        # all-gather this half of the transposed anchors (0.25 MB/rank fp8)
        if variant != "noag":
            nc.gpsimd.collective_compute(
                "AllGather",
                ALU.bypass,
                replica_groups=[list(range(N_CORES))],
                ins=[at_own[hh].opt()],
                outs=[ag[hh].opt()],
            )

    # ---- positives (overlap the AG): loads on the ACT ring.
    ptt = []
    for hh in range(2):
        (nc.gpsimd if variant == "ringiso" else nc.scalar).dma_start(
            xp_all[:, hh * 4 : (hh + 1) * 4, :],
            xp_src[:, hh * 4 : (hh + 1) * 4, :],
        )
        for rb in range(hh * 4, (hh + 1) * 4):
            sqp = scratch.tile([128, D], F32, name=f"sqp{rb}{R}", tag="sq")
            nc.vector.tensor_tensor(sqp, xp_all[:, rb, :], xp_all[:, rb, :],
                                    ALU.mult)
            nc.vector.tensor_reduce(pn_ss[:, rb : rb + 1], sqp, axis=AX.X,
                                    op=ALU.add)
    _dve_rsqrt(nc, scratch, pn_inv, pn_ss, f"p{R}")
    if True:
        for rb in range(RB):
            nc.vector.tensor_scalar_mul(
                xnp_all[:, rb, :], xp_all[:, rb, :], pn_inv[:, rb : rb + 1]
            )
            tpp = tp.tile([128, KT, 128], BF16, name=f"tpp{rb}{R}",
                          tag=f"tpp{rb % 4}")
            nc.sync.dma_start_transpose(tpp, xnp_all[:, rb, :])
            t = pin.tile([128, KT, 128], gdt, name=f"ptt{rb}{R}",
                         tag=f"ptt{rb}_{P2}")
            nc.vector.tensor_scalar_mul(t, tpp, cast_scale)
            ptt.append(t)

    # ---- diagonal: cos(pos_i, anc_i) from own normalized rows
    dprod = scratch.tile([128, RB, D], BF16, name=f"dprod{R}", tag="dprod")
    nc.vector.tensor_tensor(dprod, xna_all, xnp_all, ALU.mult)
    nc.vector.tensor_reduce(diag, dprod, axis=AX.X, op=ALU.add)

    # ---- fill gathered anchor-T tiles: atr[r] = [c, kt, 1024].
    # h0 halves of all ranks first (they only need the first AG), on the
    # SP ring so their AG waits never stall the ACT queue ahead of the exps.
    atr = [
        pin.tile([128, KT, ROWS], gdt, name=f"atr{r}{R}", tag=f"atr{r}_{P2}")
        for r in range(N_CORES)
    ]
    for hh in range(2):
        for r in range(N_CORES):
            src = at_own[hh] if variant == "noag" else ag[hh][r]
            nc.sync.dma_start(
                atr[r][:, :, hh * 512 : (hh + 1) * 512],
                src.rearrange("kt c p -> c kt p"),
            )

    if variant == "nomm":
        for r in range(N_CORES):
            nc.vector.tensor_reduce(
                ls_all[:, r, 0:1], atr[r][:, 0, 0:16], axis=AX.X, op=ALU.add,
            )
        for m in range(RB):
            nc.vector.tensor_reduce(
                ls_all[:, m, 1:2], ptt[m][:, 0, 0:16], axis=AX.X, op=ALU.add,
            )
        nc.vector.tensor_reduce(row_sum, ls_all, axis=AX.X, op=ALU.add)
    else:
        _gemm_tail(nc, R, fp8, variant, gemm_scale, bias_t, ls_all,
                   ptt, atr, psum_pool, esc_pool, row_sum=row_sum)

    # ---- per-row loss = ln(sum_j exp(logit)) - (w*diag + b)
    tail()


def build(w: float, b: float, repeat: int = 1, fp8: bool = True,
          variant: str = "full"):
    nc = bacc.Bacc(
        "TRN2",
        target_bir_lowering=False,
        debug=False,
        enable_asserts=False,
        num_devices=N_CORES,
    )
    nc._x_in = nc.dram_tensor("x_own", [2 * ROWS, D], F32, kind="ExternalInput")
    nc._out = nc.dram_tensor("out", [128, RB], F32, kind="ExternalOutput")

    with tile.TileContext(nc) as tc:
        with (
            tc.tile_pool(name="pin", bufs=1) as pin,
            tc.tile_pool(name="scratch", bufs=3) as scratch,
            tc.tile_pool(name="tp", bufs=4) as tp,
            tc.tile_pool(name="esc", bufs=3) as esc_pool,
            tc.tile_pool(name="psum", bufs=2, space="PSUM") as psum_pool,
            tc.tile_pool(name="dram", bufs=1, space="DRAM") as dram,
        ):
            pools = (pin, scratch, tp, esc_pool, psum_pool, dram)
            for rep in range(repeat):
                _emit(nc, pools, w, b, rep, fp8, variant)

    nc.compile()
    return nc


_CACHE: dict = {}


def _get_nc(w: float, b: float, repeat: int = 1, fp8: bool = True,
            variant: str = "full"):
    key = (w, b, repeat, fp8, variant)
    if key not in _CACHE:
        _CACHE[key] = build(w, b, repeat, fp8, variant)
    return _CACHE[key]


def run(x: np.ndarray, w: float, b: float, repeat: int = 1, fp8: bool = True,
        variant: str = "full", **spmd_kwargs):
    nc = _get_nc(w, b, repeat, fp8, variant)
    x = np.ascontiguousarray(np.asarray(x, dtype=np.float32))
    in_maps = [
        {"x_own": x[c * 2 * ROWS : (c + 1) * 2 * ROWS]} for c in range(N_CORES)
    ]
    res = run_bass_kernel_spmd(nc, in_maps, core_ids=list(range(N_CORES)),
                               **spmd_kwargs)
    total = sum(float(r["out"].astype(np.float64).sum()) for r in res.results)
    return np.float32(total / N_PAIRS), res


def kernel(x, w, b):
    wf = float(np.asarray(w))
    bf = float(np.asarray(b))
    loss, _ = run(np.asarray(x), wf, bf)
    return loss
